# revision 14
# baseline (speedup 1.0000x reference)
"""BinLinear TRN2 kernel: out = x @ sign(weight).T + sign(bias).

Full shapes: x [8192, 4096] f32, weight [4096, 4096] f32, bias [4096] f32
-> out [8192, 4096] f32.

Sharding (8 NeuronCores): 2D grid, 4-way over tokens x 2-way over output
features. Each core computes out_c = x_c @ sign(w_c).T + sign(b_c) with
x_c [2048, 4096], w_c [2048, 4096], b_c [2048] -> out_c [2048, 2048].
The host only slices inputs and stitches the 4x2 output grid back together.

Per-core device program (single-pass mixed fp16 x fp8 matmul, everything
on device).  The run is DMA-limited: fp32->fp16 SWDGE casts saturate the
~400GB/s DMA bus (~18us per 4.2MB two-slab chunk) and each XBAR
dma-transpose burns ~74us of DMA-engine time per MB at its fixed 256B
packet size, while the PE (437us of matmul) has slack.  So:
  - One SWDGE cast chain streams fp32->fp16 slabs of w and x DRAM->SBUF
    (first slabs singly for a fast PE start, then 256-row chunks),
    interleaved x,w,w,x,... so the PE's (w-pairs x x-slabs) work frontier
    grows quadratically while the stream is linear.
  - w slabs are transposed ON THE PE (identity-matmul through PSUM,
    [128,128] blocks) instead of the XBAR -- spending spare PE cycles to
    take ~half the transpose traffic off the saturated DMA engines.  The
    DVE binarization ((w16>0)-0.5 = 0.5*sign(w), one fused op per 8
    kt-blocks) reads the PSUM transpose result directly and writes
    resident *fp8e4* wT pair-tiles [128, 32kt, 256feat] (mixed
    fp16-stationary x fp8-moving matmul verified exact on HW; +-0.5 is
    exact in e4m3), so PE-transposing w adds zero extra copy ops.
  - x slabs go through the XBAR (now x-only) into a fp16 xT ring.
  - PE: uniform [128-token, 256-feature] PSUM cells: one K=1 matmul
    seeds the bias row (0.5-ones^T @ sign(b)), then 32 K=128 matmuls.
    N=256 runs at full PE rate and lets a cell depend on a single w
    pair-tile, so compute starts ~25us in on partially-arrived weights.
  - Strict engine separation so no in-order queue mixes early-ready and
    late-ready work: Pool=casts, SP=x XBARs, DVE=weight signs,
    Act=bias sign + psum*2 copy-back + output DMA, PE=transposes+matmuls.
"""

import sys

if "/opt/trn_rl_repo" not in sys.path:
    sys.path.insert(0, "/opt/trn_rl_repo")

from contextlib import ExitStack

import numpy as np

import concourse.bass as bass
import concourse.mybir as mybir
import concourse.tile as tile
from concourse import bacc
from concourse.bass_utils import run_bass_kernel_spmd
from concourse.masks import make_identity
from concourse.tile_rust import add_dep_helper

N_TOK, D_IN, D_OUT = 8192, 4096, 4096
TOK_WAYS, OUT_WAYS = 4, 2
N_CORES = TOK_WAYS * OUT_WAYS
TOK_SH = N_TOK // TOK_WAYS    # 2048 tokens per core
OUT_SH = D_OUT // OUT_WAYS    # 2048 out features per core

P = 128
KT = D_IN // P                # 32 contraction subtiles
NFREE = 256                   # PSUM free dim per matmul (one w pair-tile)
NSL = TOK_SH // P             # 16 token slabs
NWS = OUT_SH // P             # 16 weight slabs
NPAIR = NWS // 2              # 8 weight pair-tiles
PRO_S = 6                     # x slabs resident during the prologue
RING = PRO_S                  # xT ring size
KTG = 8                       # kt-blocks per transpose-psum group

F16 = mybir.dt.float16
F8 = mybir.dt.float8e4
F32 = mybir.dt.float32


def _build(exact_sign: bool):
    """Build the per-core SPMD program."""
    nc = bacc.Bacc("TRN2", target_bir_lowering=False, debug=False,
                   num_devices=N_CORES)
    x = nc.dram_tensor("x", [TOK_SH, D_IN], F32, kind="ExternalInput")
    w = nc.dram_tensor("w", [OUT_SH, D_IN], F32, kind="ExternalInput")
    b = nc.dram_tensor("b", [1, OUT_SH], F32, kind="ExternalInput")
    out = nc.dram_tensor("out", [TOK_SH, OUT_SH], F32, kind="ExternalOutput")

    with ExitStack() as ctx:
        tc = ctx.enter_context(tile.TileContext(nc))
        wTp = ctx.enter_context(tc.tile_pool(name="wTp", bufs=NPAIR))
        xTp = ctx.enter_context(tc.tile_pool(name="xTp", bufs=RING))
        stagep = ctx.enter_context(tc.tile_pool(name="stagep", bufs=5))
        sgtmp = ctx.enter_context(tc.tile_pool(name="sgtmp", bufs=2))
        osbp = ctx.enter_context(tc.tile_pool(name="osbp", bufs=3))
        constp = ctx.enter_context(tc.tile_pool(name="constp", bufs=1))
        mmps = ctx.enter_context(tc.tile_pool(name="mmps", bufs=6, space="PSUM"))
        wtps = ctx.enter_context(tc.tile_pool(name="wtps", bufs=2, space="PSUM"))

        def sign_half(dst_ap, src_ap, tmp_shape, tag):
            """dst(fp8) = 0.5*sign(src fp16 in PSUM), fused DVE op(s)."""
            if exact_sign:
                t1 = sgtmp.tile(tmp_shape, F16, tag=tag, name=f"{tag}_t")
                nc.vector.tensor_scalar(t1[:], src_ap, 0.0, None,
                                        mybir.AluOpType.is_lt)
                nc.vector.tensor_scalar(src_ap, src_ap, 0.0, None,
                                        mybir.AluOpType.is_gt)
                nc.vector.tensor_tensor(src_ap, src_ap, t1[:],
                                        mybir.AluOpType.subtract)
                nc.vector.tensor_scalar(dst_ap, src_ap, 0.5, None,
                                        mybir.AluOpType.mult)
            else:
                nc.vector.tensor_scalar(
                    dst_ap, src_ap, 0.0, 0.5,
                    mybir.AluOpType.is_gt, mybir.AluOpType.subtract,
                )

        # ---- SWDGE cast chain (DRAM fp32 -> SBUF fp16), nosync-ordered so
        # chunks complete in stream order.
        last_swdge = [None]

        def swdge_cast(dst_ap, src_ap):
            inst = nc.gpsimd.dma_start(dst_ap, src_ap)
            if last_swdge[0] is not None:
                add_dep_helper(inst.ins, last_swdge[0].ins, sync=False,
                               reason="SWDGE cast order")
            last_swdge[0] = inst
            return inst

        # ---- constants + bias row: brow = sign(b) (+-1, exact) on Act;
        # the K=1 bias seed matmul uses 0.5-valued ones so psum gets
        # 0.5*sign(b), matching the 0.5*sign(w) accumulation, x2 on copy.
        ones = constp.tile([1, P], F16)
        nc.gpsimd.memset(ones[:], 0.5)
        ident = constp.tile([P, P], F16)
        make_identity(nc, ident[:])
        b16 = constp.tile([1, OUT_SH], F16)
        swdge_cast(b16[:], b[:])
        brow = constp.tile([1, OUT_SH], F16)
        nc.scalar.activation(brow[:], b16[:],
                             mybir.ActivationFunctionType.Sign)

        # ---- resident weight pair-tiles (fp8) and the xT ring (fp16)
        wT = [wTp.tile([P, KT, NFREE], F8, tag="wT", name=f"wT{q}")
              for q in range(NPAIR)]
        xT = [None] * NSL

        def cast_chunk(src, j0, nsl, name):
            """Cast rows [j0*P, (j0+nsl)*P) of src into an nsl-slab stage."""
            st = stagep.tile([P, nsl, D_IN], F16, tag="stage", name=name)
            src_ap = src[j0 * P:(j0 + nsl) * P, :]
            if nsl > 1:
                src_ap = src_ap.rearrange("(a p) d -> p a d", p=P)
                swdge_cast(st[:], src_ap)
            else:
                swdge_cast(st[:, 0, :], src_ap)
            return st

        wstage = [None] * NWS

        def w_cast(j0, nsl):
            """Cast weight slabs [j0, j0+nsl) into a stage chunk."""
            st = cast_chunk(w, j0, nsl, f"wst{j0}")
            for a in range(nsl):
                wstage[j0 + a] = (st, a)

        def w_transpose(j):
            """PE-transpose staged weight slab j ([128,128] blocks through
            PSUM), fused DVE sign-cast into the fp8 pair-tile."""
            st, a = wstage[j]
            q, jj = j // 2, j % 2
            for g in range(KT // KTG):
                pt = wtps.tile([P, KTG, P], F16, tag="wtp", name="wtp")
                for m in range(KTG):
                    kt = g * KTG + m
                    nc.tensor.transpose(
                        pt[:, m, :],
                        st[:, a, kt * P:(kt + 1) * P], ident[:])
                sign_half(
                    wT[q][:, g * KTG:(g + 1) * KTG, jj * P:(jj + 1) * P],
                    pt[:], [P, KTG, P], "wsg")

        def x_slabs(s0, nsl):
            """Stream token slabs [s0, s0+nsl): cast, XBAR into the ring.
            Each slab is transposed in 4 quarter-XBARs so the DMA engines
            can interleave cast descriptors between the XBAR packet
            bursts instead of stalling the cast stream for a full slab."""
            st = cast_chunk(x, s0, nsl, f"xst{s0}")
            for a in range(nsl):
                s = s0 + a
                xT[s] = xTp.tile([P, KT, P], F16, tag="xT", name=f"xT{s}")
                kq = KT // 4
                for g in range(4):
                    nc.sync.dma_start_transpose(
                        xT[s][:, g * kq:(g + 1) * kq, :],
                        st[:, a, g * kq * P:(g + 1) * kq * P])

        def cell(q, s):
            """One [128-token, 256-feature] output cell: bias seed + 32
            matmuls (fp16 stationary x, fp8 moving w), Act-engine x2
            copy-back, DMA out."""
            psum = mmps.tile([P, NFREE], F32, tag="mm", name="psum")
            nc.tensor.matmul(
                psum[:], ones[:], brow[0:1, q * NFREE:(q + 1) * NFREE],
                start=True, stop=False,
            )
            for kt in range(KT):
                nc.tensor.matmul(
                    psum[:], xT[s][:, kt, :], wT[q][:, kt, :],
                    start=False, stop=(kt == KT - 1),
                )
            osb = osbp.tile([P, NFREE], F32, tag="osb", name="osb")
            nc.scalar.activation(osb[:], psum[:],
                                 mybir.ActivationFunctionType.Copy, 0.0, 2.0)
            nc.scalar.dma_start(
                out[s * P:(s + 1) * P, q * NFREE:(q + 1) * NFREE], osb[:])

        # ---- software-pipelined prologue: casts run ~2 rounds ahead;
        # each round runs the transposes of the chunk that just landed
        # BEFORE the cells of already-transposed pairs, so w stages are
        # released one round early and the cast chain never blocks on PE
        # progress through cell batches.
        x_slabs(0, 1)
        w_cast(0, 1)
        w_cast(1, 1)
        x_slabs(1, 1)
        w_cast(2, 2)
        w_transpose(0)
        w_transpose(1)
        cell(0, 0)
        cell(0, 1)
        x_slabs(2, 2)
        w_transpose(2)
        w_transpose(3)
        cell(1, 0)
        cell(1, 1)
        w_cast(4, 2)
        for q, s in ((0, 2), (0, 3), (1, 2), (1, 3)):
            cell(q, s)
        x_slabs(4, 2)
        w_transpose(4)
        w_transpose(5)
        for s in range(4):
            cell(2, s)
        w_cast(6, 2)
        w_transpose(6)
        w_transpose(7)
        for s in range(4):
            cell(3, s)
        w_cast(8, 2)
        w_transpose(8)
        w_transpose(9)
        for q, s in ((0, 4), (0, 5), (1, 4), (1, 5)):
            cell(q, s)
        w_cast(10, 2)
        w_transpose(10)
        w_transpose(11)
        for q, s in ((2, 4), (2, 5), (3, 4), (3, 5)):
            cell(q, s)
        w_cast(12, 2)
        w_transpose(12)
        w_transpose(13)
        for s in range(PRO_S):
            cell(4, s)
        w_cast(14, 2)
        w_transpose(14)
        w_transpose(15)
        for s in range(PRO_S):
            cell(5, s)
        x_slabs(6, 2)
        for s in range(PRO_S):
            cell(6, s)
        x_slabs(8, 2)
        for s in range(PRO_S):
            cell(7, s)

        # ---- bulk: remaining token slabs, slab-major (wT fully resident).
        for s0 in range(PRO_S, NSL, 2):
            if s0 + 4 < NSL:
                x_slabs(s0 + 4, 2)
            for s in (s0, s0 + 1):
                for q in range(NPAIR):
                    cell(q, s)

    nc.finalize()
    return nc


_cache = {}


def _get_nc(exact_sign: bool):
    if exact_sign not in _cache:
        _cache[exact_sign] = _build(exact_sign)
    return _cache[exact_sign]


def kernel(x: np.ndarray, weight: np.ndarray, bias: np.ndarray) -> np.ndarray:
    x = np.ascontiguousarray(np.asarray(x, dtype=np.float32))
    weight = np.ascontiguousarray(np.asarray(weight, dtype=np.float32))
    bias = np.ascontiguousarray(np.asarray(bias, dtype=np.float32))
    assert x.shape == (N_TOK, D_IN) and weight.shape == (D_OUT, D_IN)

    # (w > 0) - 0.5 equals 0.5*sign(w) only when no exact zeros exist;
    # fall back to the exact 3-op sign variant otherwise (bias zeros are
    # handled exactly by the Act-engine Sign either way).
    exact_sign = bool((weight == 0.0).any())
    nc = _get_nc(exact_sign)

    in_maps = []
    for tg in range(TOK_WAYS):
        for og in range(OUT_WAYS):
            in_maps.append({
                "x": np.ascontiguousarray(x[tg * TOK_SH:(tg + 1) * TOK_SH, :]),
                "w": np.ascontiguousarray(weight[og * OUT_SH:(og + 1) * OUT_SH, :]),
                "b": np.ascontiguousarray(
                    bias[og * OUT_SH:(og + 1) * OUT_SH].reshape(1, OUT_SH)),
            })

    res = run_bass_kernel_spmd(nc, in_maps, list(range(N_CORES)))

    out = np.empty((N_TOK, D_OUT), dtype=np.float32)
    c = 0
    for tg in range(TOK_WAYS):
        for og in range(OUT_WAYS):
            out[tg * TOK_SH:(tg + 1) * TOK_SH, og * OUT_SH:(og + 1) * OUT_SH] = \
                res.results[c]["out"]
            c += 1
    return out


# revision 19
# speedup vs baseline: 1.2925x; 1.2925x over previous
"""BinLinear TRN2 kernel: out = x @ sign(weight).T + sign(bias).

Full shapes: x [8192, 4096] f32, weight [4096, 4096] f32, bias [4096] f32
-> out [8192, 4096] f32.

Sharding (8 NeuronCores): 2D grid, 4-way over tokens x 2-way over output
features. Each core computes out_c = x_c @ sign(w_c).T + sign(b_c) with
x_c [2048, 4096], w_c [2048, 4096], b_c [2048] -> out_c [2048, 2048].
The host only slices inputs and stitches the 4x2 output grid back together.

Per-core device program (single-pass mixed fp16 x fp8 matmul, everything
on device).  The run is DMA-limited: fp32->fp16 SWDGE casts saturate the
~400GB/s DMA bus (~18us per 4.2MB two-slab chunk) and each XBAR
dma-transpose burns ~74us of DMA-engine time per MB at its fixed 256B
packet size, while the PE (437us of matmul) has slack.  So:
  - One SWDGE cast chain streams fp32->fp16 slabs of w and x DRAM->SBUF
    (first slabs singly for a fast PE start, then 256-row chunks),
    interleaved x,w,w,x,... so the PE's (w-pairs x x-slabs) work frontier
    grows quadratically while the stream is linear.
  - w slabs are transposed ON THE PE (identity-matmul through PSUM,
    [128,128] blocks) instead of the XBAR -- spending spare PE cycles to
    take ~half the transpose traffic off the saturated DMA engines.  The
    DVE binarization ((w16>0)-0.5 = 0.5*sign(w), one fused op per 8
    kt-blocks) reads the PSUM transpose result directly and writes
    resident *fp8e4* wT pair-tiles [128, 32kt, 256feat] (mixed
    fp16-stationary x fp8-moving matmul verified exact on HW; +-0.5 is
    exact in e4m3), so PE-transposing w adds zero extra copy ops.
  - x slabs go through the XBAR (now x-only) into a fp16 xT ring.
  - PE: uniform [128-token, 256-feature] PSUM cells: one K=1 matmul
    seeds the bias row (0.5-ones^T @ sign(b)), then 32 K=128 matmuls.
    N=256 runs at full PE rate and lets a cell depend on a single w
    pair-tile, so compute starts ~25us in on partially-arrived weights.
  - Strict engine separation so no in-order queue mixes early-ready and
    late-ready work: Pool=casts, SP=x XBARs, DVE=weight signs,
    Act=bias sign + psum*2 copy-back + output DMA, PE=transposes+matmuls.
"""

import sys

if "/opt/trn_rl_repo" not in sys.path:
    sys.path.insert(0, "/opt/trn_rl_repo")

from contextlib import ExitStack

import numpy as np

import concourse.bass as bass
import concourse.mybir as mybir
import concourse.tile as tile
from concourse import bacc
from concourse.bass_utils import run_bass_kernel_spmd
from concourse.masks import make_identity
from concourse.tile_rust import add_dep_helper

N_TOK, D_IN, D_OUT = 8192, 4096, 4096
TOK_WAYS, OUT_WAYS = 4, 2
N_CORES = TOK_WAYS * OUT_WAYS
TOK_SH = N_TOK // TOK_WAYS    # 2048 tokens per core
OUT_SH = D_OUT // OUT_WAYS    # 2048 out features per core

P = 128
KT = D_IN // P                # 32 contraction subtiles
NFREE = 256                   # PSUM free dim per matmul (one w pair-tile)
NSL = TOK_SH // P             # 16 token slabs
NWS = OUT_SH // P             # 16 weight slabs
NPAIR = NWS // 2              # 8 weight pair-tiles
PRO_S = 6                     # x slabs resident during the prologue
RING = PRO_S                  # xT ring size
KTG = 8                       # kt-blocks per transpose-psum group

F16 = mybir.dt.float16
F8 = mybir.dt.float8e4
F32 = mybir.dt.float32


def _build(exact_sign: bool):
    """Build the per-core SPMD program."""
    nc = bacc.Bacc("TRN2", target_bir_lowering=False, debug=False,
                   num_devices=N_CORES)
    x = nc.dram_tensor("x", [TOK_SH, D_IN], F32, kind="ExternalInput")
    w = nc.dram_tensor("w", [OUT_SH, D_IN], F32, kind="ExternalInput")
    b = nc.dram_tensor("b", [1, OUT_SH], F32, kind="ExternalInput")
    out = nc.dram_tensor("out", [TOK_SH, OUT_SH], F32, kind="ExternalOutput")

    with ExitStack() as ctx:
        tc = ctx.enter_context(tile.TileContext(nc))
        wTp = ctx.enter_context(tc.tile_pool(name="wTp", bufs=NPAIR))
        xTp = ctx.enter_context(tc.tile_pool(name="xTp", bufs=RING))
        stagep = ctx.enter_context(tc.tile_pool(name="stagep", bufs=5))
        sgtmp = ctx.enter_context(tc.tile_pool(name="sgtmp", bufs=2))
        osbp = ctx.enter_context(tc.tile_pool(name="osbp", bufs=3))
        constp = ctx.enter_context(tc.tile_pool(name="constp", bufs=1))
        mmps = ctx.enter_context(tc.tile_pool(name="mmps", bufs=6, space="PSUM"))
        wtps = ctx.enter_context(tc.tile_pool(name="wtps", bufs=2, space="PSUM"))

        def sign_half(dst_ap, src_ap, tmp_shape, tag):
            """dst(fp8) = 0.5*sign(src fp16 in PSUM), fused DVE op(s)."""
            if exact_sign:
                t1 = sgtmp.tile(tmp_shape, F16, tag=tag, name=f"{tag}_t")
                nc.vector.tensor_scalar(t1[:], src_ap, 0.0, None,
                                        mybir.AluOpType.is_lt)
                nc.vector.tensor_scalar(src_ap, src_ap, 0.0, None,
                                        mybir.AluOpType.is_gt)
                nc.vector.tensor_tensor(src_ap, src_ap, t1[:],
                                        mybir.AluOpType.subtract)
                nc.vector.tensor_scalar(dst_ap, src_ap, 0.5, None,
                                        mybir.AluOpType.mult)
            else:
                nc.vector.tensor_scalar(
                    dst_ap, src_ap, 0.0, 0.5,
                    mybir.AluOpType.is_gt, mybir.AluOpType.subtract,
                )

        # ---- SWDGE cast chain (DRAM fp32 -> SBUF fp16), nosync-ordered so
        # chunks complete in stream order.
        last_swdge = [None]

        def swdge_cast(dst_ap, src_ap):
            inst = nc.gpsimd.dma_start(dst_ap, src_ap)
            if last_swdge[0] is not None:
                add_dep_helper(inst.ins, last_swdge[0].ins, sync=False,
                               reason="SWDGE cast order")
            last_swdge[0] = inst
            return inst

        # ---- constants + bias row: brow = sign(b) (+-1, exact) on Act;
        # the K=1 bias seed matmul uses 0.5-valued ones so psum gets
        # 0.5*sign(b), matching the 0.5*sign(w) accumulation, x2 on copy.
        ones = constp.tile([1, P], F16)
        nc.gpsimd.memset(ones[:], 0.5)
        ident = constp.tile([P, P], F16)
        make_identity(nc, ident[:])
        b16 = constp.tile([1, OUT_SH], F16)
        swdge_cast(b16[:], b[:])
        brow = constp.tile([1, OUT_SH], F16)
        nc.scalar.activation(brow[:], b16[:],
                             mybir.ActivationFunctionType.Sign)

        # ---- resident weight pair-tiles (fp8) and the xT ring (fp16)
        wT = [wTp.tile([P, KT, NFREE], F8, tag="wT", name=f"wT{q}")
              for q in range(NPAIR)]
        xT = [None] * NSL

        def cast_chunk(src, j0, nsl, name):
            """Cast rows [j0*P, (j0+nsl)*P) of src into an nsl-slab stage."""
            st = stagep.tile([P, nsl, D_IN], F16, tag="stage", name=name)
            src_ap = src[j0 * P:(j0 + nsl) * P, :]
            if nsl > 1:
                src_ap = src_ap.rearrange("(a p) d -> p a d", p=P)
                swdge_cast(st[:], src_ap)
            else:
                swdge_cast(st[:, 0, :], src_ap)
            return st

        wstage = [None] * NWS

        def w_cast(j0, nsl):
            """Cast weight slabs [j0, j0+nsl) into a stage chunk."""
            st = cast_chunk(w, j0, nsl, f"wst{j0}")
            for a in range(nsl):
                wstage[j0 + a] = (st, a)

        def w_transpose(j):
            """PE-transpose staged weight slab j ([128,128] blocks through
            PSUM), fused DVE sign-cast into the fp8 pair-tile."""
            st, a = wstage[j]
            q, jj = j // 2, j % 2
            for g in range(KT // KTG):
                pt = wtps.tile([P, KTG, P], F16, tag="wtp", name="wtp")
                for m in range(KTG):
                    kt = g * KTG + m
                    nc.tensor.transpose(
                        pt[:, m, :],
                        st[:, a, kt * P:(kt + 1) * P], ident[:])
                sign_half(
                    wT[q][:, g * KTG:(g + 1) * KTG, jj * P:(jj + 1) * P],
                    pt[:], [P, KTG, P], "wsg")

        def x_slabs(s0, nsl):
            """Stream token slabs [s0, s0+nsl): cast, XBAR into the ring."""
            st = cast_chunk(x, s0, nsl, f"xst{s0}")
            for a in range(nsl):
                s = s0 + a
                xT[s] = xTp.tile([P, KT, P], F16, tag="xT", name=f"xT{s}")
                nc.sync.dma_start_transpose(xT[s][:], st[:, a, :])

        xstage = [None] * NSL

        def x_cast(s0, nsl):
            """Cast token slabs [s0, s0+nsl) into a stage chunk."""
            st = cast_chunk(x, s0, nsl, f"xst{s0}")
            for a in range(nsl):
                xstage[s0 + a] = (st, a)

        def x_transpose_pe(s):
            """Prologue variant: transpose staged token slab s on the PE
            (through PSUM, DVE copy-back) instead of the XBAR, keeping the
            DMA engines free for the weight cast stream during the
            critical fill window."""
            st, a = xstage[s]
            xT[s] = xTp.tile([P, KT, P], F16, tag="xT", name=f"xT{s}")
            for g in range(KT // KTG):
                pt = wtps.tile([P, KTG, P], F16, tag="wtp", name="wtp")
                for m in range(KTG):
                    kt = g * KTG + m
                    nc.tensor.transpose(
                        pt[:, m, :],
                        st[:, a, kt * P:(kt + 1) * P], ident[:])
                nc.vector.tensor_copy(
                    xT[s][:, g * KTG:(g + 1) * KTG, :], pt[:])

        def cell(q, s):
            """One [128-token, 256-feature] output cell: bias seed + 32
            matmuls (fp16 stationary x, fp8 moving w), Act-engine x2
            copy-back, DMA out."""
            psum = mmps.tile([P, NFREE], F32, tag="mm", name="psum")
            nc.tensor.matmul(
                psum[:], ones[:], brow[0:1, q * NFREE:(q + 1) * NFREE],
                start=True, stop=False,
            )
            for kt in range(KT):
                nc.tensor.matmul(
                    psum[:], xT[s][:, kt, :], wT[q][:, kt, :],
                    start=False, stop=(kt == KT - 1),
                )
            osb = osbp.tile([P, NFREE], F32, tag="osb", name="osb")
            nc.scalar.activation(osb[:], psum[:],
                                 mybir.ActivationFunctionType.Copy, 0.0, 2.0)
            nc.scalar.dma_start(
                out[s * P:(s + 1) * P, q * NFREE:(q + 1) * NFREE], osb[:])

        # ---- software-pipelined prologue: casts run ~2 rounds ahead;
        # each round runs the transposes of the chunk that just landed
        # BEFORE the cells of already-transposed pairs, so w stages are
        # released one round early and the cast chain never blocks on PE
        # progress through cell batches.
        x_cast(0, 1)
        w_cast(0, 1)
        w_cast(1, 1)
        x_cast(1, 1)
        w_cast(2, 2)
        x_transpose_pe(0)
        w_transpose(0)
        w_transpose(1)
        x_transpose_pe(1)
        cell(0, 0)
        cell(0, 1)
        x_cast(2, 2)
        w_transpose(2)
        w_transpose(3)
        x_transpose_pe(2)
        cell(1, 0)
        cell(1, 1)
        w_cast(4, 2)
        x_transpose_pe(3)
        for q, s in ((0, 2), (0, 3), (1, 2), (1, 3)):
            cell(q, s)
        x_cast(4, 2)
        w_transpose(4)
        w_transpose(5)
        x_transpose_pe(4)
        x_transpose_pe(5)
        for s in range(4):
            cell(2, s)
        w_cast(6, 2)
        w_transpose(6)
        w_transpose(7)
        for s in range(4):
            cell(3, s)
        w_cast(8, 2)
        w_transpose(8)
        w_transpose(9)
        for q, s in ((0, 4), (0, 5), (1, 4), (1, 5)):
            cell(q, s)
        w_cast(10, 2)
        w_transpose(10)
        w_transpose(11)
        for q, s in ((2, 4), (2, 5), (3, 4), (3, 5)):
            cell(q, s)
        w_cast(12, 2)
        w_transpose(12)
        w_transpose(13)
        for s in range(PRO_S):
            cell(4, s)
        w_cast(14, 2)
        w_transpose(14)
        w_transpose(15)
        for s in range(PRO_S):
            cell(5, s)
        x_slabs(6, 2)
        for s in range(PRO_S):
            cell(6, s)
        x_slabs(8, 2)
        for s in range(PRO_S):
            cell(7, s)

        # ---- bulk: remaining token slabs, slab-major (wT fully resident).
        for s0 in range(PRO_S, NSL, 2):
            if s0 + 4 < NSL:
                x_slabs(s0 + 4, 2)
            for s in (s0, s0 + 1):
                for q in range(NPAIR):
                    cell(q, s)

    nc.finalize()
    return nc


_cache = {}


def _get_nc(exact_sign: bool):
    if exact_sign not in _cache:
        _cache[exact_sign] = _build(exact_sign)
    return _cache[exact_sign]


def kernel(x: np.ndarray, weight: np.ndarray, bias: np.ndarray) -> np.ndarray:
    x = np.ascontiguousarray(np.asarray(x, dtype=np.float32))
    weight = np.ascontiguousarray(np.asarray(weight, dtype=np.float32))
    bias = np.ascontiguousarray(np.asarray(bias, dtype=np.float32))
    assert x.shape == (N_TOK, D_IN) and weight.shape == (D_OUT, D_IN)

    # (w > 0) - 0.5 equals 0.5*sign(w) only when no exact zeros exist;
    # fall back to the exact 3-op sign variant otherwise (bias zeros are
    # handled exactly by the Act-engine Sign either way).
    exact_sign = bool((weight == 0.0).any())
    nc = _get_nc(exact_sign)

    in_maps = []
    for tg in range(TOK_WAYS):
        for og in range(OUT_WAYS):
            in_maps.append({
                "x": np.ascontiguousarray(x[tg * TOK_SH:(tg + 1) * TOK_SH, :]),
                "w": np.ascontiguousarray(weight[og * OUT_SH:(og + 1) * OUT_SH, :]),
                "b": np.ascontiguousarray(
                    bias[og * OUT_SH:(og + 1) * OUT_SH].reshape(1, OUT_SH)),
            })

    res = run_bass_kernel_spmd(nc, in_maps, list(range(N_CORES)))

    out = np.empty((N_TOK, D_OUT), dtype=np.float32)
    c = 0
    for tg in range(TOK_WAYS):
        for og in range(OUT_WAYS):
            out[tg * TOK_SH:(tg + 1) * TOK_SH, og * OUT_SH:(og + 1) * OUT_SH] = \
                res.results[c]["out"]
            c += 1
    return out
